# revision 13
# baseline (speedup 1.0000x reference)
"""Trainium2 Bass kernel for the mixture log-likelihood loss.

reference:
    log_otu = log(otu_dist + EPS)                       # (K=64, O=1024)
    lse[n,k] = counts[n] . log_otu[k] + log(comm+EPS)[k]
    out = sum_n logsumexp_k(lse[n, :])

Strategy (8 NeuronCores, data-parallel over N):
  * counts rows are small integers -> exact in fp8 e4m3. Cast on host,
    quartering HBM traffic (the kernel is memory-bound on counts). Falls
    back to an identically-structured bf16 module if the data ever stops
    being fp8-exact.
  * log_otu is quantized to a single fp8 operand (no hi/lo split): the
    per-weight quantization error averages out over the 1024-wide dot
    product; measured end-to-end relative error ~1e-3 against the 2e-2
    budget. This halves the matmul free dim (64 instead of 128), which
    halves PE stream time -- the kernel-wide bottleneck.
  * The per-community prior log(comm+EPS) is added by the PE itself: a
    1-partition bf16 matmul (ones[1,128] x prior[1,4*64]) opens each
    PSUM accumulation group, so the vector engine never has to touch the
    logits for an elementwise add.
  * Eight 128-particle blocks share one full 2KB PSUM bank (128, 8, 64).
    Per octet: one DVE reduce_max(negate) from PSUM, one DVE broadcast-add
    of -max (stride-0 AP along k), ONE batched exp on the scalar engine
    (no per-block bias instructions), one bf16 DVE reduce_sum. Batching at
    bank granularity halves the per-element share of the fixed PSUM access
    cost (~125ns DVE / ~185ns ACT per instruction).
    (Measured dead ends: activation accum_out emits a ~378ns
    READ_ACCUMULATOR per op; InstTensorTensorReduce crashes the HW NEFF;
    gpsimd tensor_add fails walrus codegen; gpsimd reduce only does
    partition-axis.)
  * All Ln work is deferred to segment boundaries.
  * Per-core partial sum is reduced over partitions with a tiny f32
    matmul against ones; the host adds the 8 scalars and analytically
    removes the zero-row padding contribution.
"""

import numpy as np
import ml_dtypes

N, K, O = 100000, 64, 1024
EPS = 1e-6
CORES = 8
NSHARD = N // CORES          # 12500
BLK = 128
NBLK = 98                    # ceil(12500 / 128)
NPAD = NBLK * BLK            # 12544
BPS = 14                     # blocks per superblock (even: pairs don't split)
SBS = NBLK // BPS            # 7 superblocks
PAD_ROWS = NPAD - NSHARD     # 44 zero rows per core

_cache = {}


def _build_module(use_fp8):
    import os
    F_CDMA = os.environ.get("K_CDMA", "a")   # a=scalar HWDGE, g=gpsimd SWDGE
    NWARM = int(os.environ.get("K_WARM", "55"))  # HAM warmup matmuls
    SPLIT_FROM = int(os.environ.get("K_SPLIT", "6"))  # counts pieces >= this
    # index alternate onto the scalar HWDGE ring (-1 = sync ring only)
    F_PRIORMM = os.environ.get("K_PRIORMM", "1") == "1"
    F_ACCUM = os.environ.get("K_ACCUM", "0") == "1"  # READ_ACCUMULATOR costs ~378ns/op on ACT
    F_TTR = os.environ.get("K_TTR", "0") == "1"  # InstTensorTensorReduce crashes HW NEFF
    F_UPFRONT = os.environ.get("K_UPFRONT", "1") == "1"
    F_BEXP = os.environ.get("K_BEXP", "1") == "1"    # batched exp per quad
    F_GSUM = os.environ.get("K_GSUM", "0") == "1"    # gpsimd reduce: axis X unsupported
    ADDE = os.environ.get("K_ADDE", "v")             # max-subtract engine: v=DVE (gpsimd fails walrus codegen)
    import concourse.bacc as bacc
    import concourse.tile as tile
    from concourse import mybir

    # Force all activations (Exp/Ln/Copy) onto the one ACT table set that
    # contains them all — otherwise every Exp<->Ln switch pays a ~1.3us
    # ACT_TABLE_LOAD. Other sets are blanked (positions kept so the
    # act_func_set_id -> act_info.json index mapping stays valid).
    if not getattr(bacc, "_act_tables_patched", False):
        _orig_get = bacc.get_activation_tables

        def _only_ln_exp(arch):
            tabs = _orig_get(arch)
            return {
                name: (fns if name == "natural_log_exp_and_others" else set())
                for name, fns in tabs.items()
            }

        bacc.get_activation_tables = _only_ln_exp
        bacc._act_tables_patched = True

    f32 = mybir.dt.float32
    bf16 = mybir.dt.bfloat16
    cdt = mybir.dt.float8e4 if use_fp8 else bf16
    AX = mybir.AxisListType.X
    AF = mybir.ActivationFunctionType
    ALU = mybir.AluOpType

    nc = bacc.Bacc("TRN2", target_bir_lowering=False, debug=False,
                   num_devices=CORES)
    cnts = nc.dram_tensor("cnts", [SBS, 128, BPS * 8 * BLK], cdt,
                          kind="ExternalInput").ap()
    hi8 = nc.dram_tensor("hi8", [128, 8 * K], cdt,
                         kind="ExternalInput").ap()
    # prior duplicated for the 8-block batch, in the counts dtype so the
    # prior matmul never switches the PE array out of fp8 mode
    prior = nc.dram_tensor("prior", [1, 8, K], cdt,
                           kind="ExternalInput").ap()
    out = nc.dram_tensor("out", [1, 1], f32, kind="ExternalOutput").ap()

    cnt_bufs = SBS if use_fp8 else 4

    with tile.TileContext(nc, num_cores=CORES) as tc:
        with (
            tc.tile_pool(name="const", bufs=1) as const,
            tc.tile_pool(name="cnt", bufs=cnt_bufs) as cnt_pool,
            tc.tile_pool(name="work", bufs=6) as work,
            tc.tile_pool(name="psum", bufs=6, space="PSUM") as psum_pool,
            tc.tile_pool(name="fps", bufs=1, space="PSUM") as fps_pool,
            tc.tile_pool(name="wps", bufs=1, space="PSUM") as wps_pool,
        ):
            # constants ride the second HWDGE ring (ACT/scalar) -- the
            # gpsimd SWDGE path serializes ~10us of Q7 descriptor work at
            # the head, delaying the first matmul (measured); the scalar
            # ring finishes these ~65KB in <1us and the big counts DMAs
            # still own the sync HWDGE ring
            cdma = nc.scalar if F_CDMA == "a" else nc.gpsimd
            # prior first: the octet-0 prior matmul is the first real PE op
            # and the DMA completion sem only fires ~2us after last byte
            prior_sb = const.tile([1, 8, K], cdt)
            cdma.dma_start(out=prior_sb[:], in_=prior)
            hi_sb = const.tile([128, 8 * K], cdt)
            cdma.dma_start(out=hi_sb[:], in_=hi8)
            # ones vectors need no DMA at all
            onesb_sb = const.tile([1, BLK], cdt)
            nc.vector.memset(onesb_sb[:], 1.0)
            ones_sb = const.tile([128, 1], f32)
            nc.vector.memset(ones_sb[:], 1.0)
            if not F_PRIORMM:
                prior4 = nc.dram_tensor("prior4", [128, 8, K], f32,
                                        kind="ExternalInput").ap()
                prior4_sb = const.tile([128, 8, K], f32)
                cdma.dma_start(out=prior4_sb[:], in_=prior4)
            mg_all = const.tile([128, NBLK], f32)
            # bf16 so the DVE sum-exp reduce runs in 2x mode (f32 out would
            # disable the 2-byte fast path); ln(sg) tolerates 0.4% easily
            sg_all = const.tile([128, NBLK], bf16)
            # touch Exp and Ln once (into a slice that is later fully
            # overwritten, so DCE keeps it) so both ACT table loads overlap
            # the DMA-bound head instead of landing in the kernel tail
            warm = const.tile([1, 1], f32)
            nc.vector.memset(warm[:], 1.0)
            nc.scalar.activation(sg_all[0:1, 0:1], warm[:], AF.Exp)
            nc.scalar.activation(sg_all[0:1, 0:1], warm[:], AF.Ln)
            # HAM warmup: the PE clock-gate stays at 4/8 (1.2 GHz) until
            # ~3.4us of sustained matmul busy. The PE is otherwise idle
            # while the first counts land, so burn that window on dummy
            # matmuls over a memset tile; real matmuls then start warm
            # (measured 35ns/MM warm vs 64-107ns cold).
            if NWARM:
                wt = const.tile([128, BLK], cdt)
                nc.vector.memset(wt[:], 1.0)
                wps = wps_pool.tile([128, BLK], f32)
                for _ in range(NWARM):
                    nc.tensor.matmul(wps[:], lhsT=wt[:], rhs=wt[:],
                                     start=True, stop=True,
                                     skip_group_check=True)
                # consume one element so DCE keeps the warmup chain; the
                # slice is fully overwritten later
                nc.scalar.copy(sg_all[0:1, 0:1], wps[0:1, 0:1])

            BW = 8 * BLK                            # cols per block
            cnt_tiles = [None] * SBS
            piece_idx = [0]

            def load_sb(s):
                cnt = cnt_pool.tile([128, BPS * BW], cdt)
                # fine-grained first/last superblocks so the PE start and
                # the kernel drain chase the DMA closely
                if s == 0:
                    splits = [0, 2, 4, 8, BPS]
                elif s == SBS - 1:
                    # extra-fine end: the consuming matmuls wait on each
                    # piece's completion sem (~2us after last byte), so the
                    # last piece should be tiny
                    splits = [0, 4, 8, 11, 13, BPS]
                else:
                    splits = [0, 7, BPS]
                for a, b in zip(splits, splits[1:]):
                    # alternate later pieces onto the scalar HWDGE ring:
                    # two rings drain the 16 SDMA engines better than one
                    # (measured 321 GB/s single-ring vs ~358 HBM limit)
                    p = piece_idx[0]
                    piece_idx[0] += 1
                    eng = nc.scalar if (SPLIT_FROM >= 0 and p >= SPLIT_FROM
                                        and (p - SPLIT_FROM) % 2 == 0) else nc.sync
                    eng.dma_start(out=cnt[:, a * BW:b * BW],
                                  in_=cnts[s, :, a * BW:b * BW])
                cnt_tiles[s] = cnt

            def block_ap(b):
                return cnt_tiles[b // BPS], (b % BPS) * BW

            if use_fp8 and F_UPFRONT:
                for s in range(SBS):
                    load_sb(s)
            else:
                load_sb(0)

            # octets in the steady state (one full 2KB PSUM bank each);
            # quads + a pair at the end so the final serial
            # max->add->exp->sum drain chains are short
            groups = [(8 * q, 8) for q in range(10)]
            groups += [(80, 4), (84, 4), (88, 4), (92, 4), (96, 2)]
            # incremental logsumexp tail: ln/sub/sum as columns complete;
            # keep the last seg tiny so little Ln work lands in the drain
            segs = [(0, 24), (24, 48), (48, 72), (72, 88), (88, NBLK)]
            ls = const.tile([128, NBLK], f32)
            t3 = const.tile([128, NBLK], f32)
            acc8 = const.tile([128, len(segs)], f32)
            seg_after = {b: i for i, (a, b) in enumerate(segs)}

            def emit_seg(i):
                a, b = segs[i]
                nc.scalar.activation(ls[:, a:b], sg_all[:, a:b], AF.Ln)
                if F_TTR:
                    nc.vector.tensor_tensor_reduce(
                        out=t3[:, a:b], in0=ls[:, a:b], in1=mg_all[:, a:b],
                        scale=1.0, scalar=0.0, op0=ALU.subtract, op1=ALU.add,
                        accum_out=acc8[:, i:i + 1])
                else:
                    nc.vector.tensor_sub(t3[:, a:b], ls[:, a:b], mg_all[:, a:b])
                    nc.vector.reduce_sum(acc8[:, i:i + 1], t3[:, a:b], axis=AX)

            for g0, gn in groups:                   # quads of blocks
                if not (use_fp8 and F_UPFRONT):
                    for b in range(g0, g0 + gn):
                        s = b // BPS
                        if s + 1 < SBS and cnt_tiles[s + 1] is None:
                            load_sb(s + 1)
                B4 = psum_pool.tile([128, 8, K], mybir.dt.float32)
                # prior lands first via a 1-partition bf16 matmul; it opens
                # the accumulation group (start=True resets has_written)
                if F_PRIORMM:
                    nc.tensor.matmul(B4[:, :gn, :], lhsT=onesb_sb[:],
                                     rhs=prior_sb[:, :gn, :],
                                     start=True, stop=False,
                                     skip_group_check=True)
                for j in range(gn):                 # block within quad
                    tile_b, off = block_ap(g0 + j)
                    for c in range(8):
                        nc.tensor.matmul(
                            B4[:, j, :],
                            lhsT=tile_b[:, off + c * BLK:off + (c + 1) * BLK],
                            rhs=hi_sb[:, c * K:(c + 1) * K],
                            start=(not F_PRIORMM) and j == 0 and c == 0,
                            stop=(j == gn - 1 and c == 7),
                            skip_group_check=True,
                        )
                src4 = B4
                if not F_PRIORMM:
                    t4 = work.tile([128, 8, K], mybir.dt.float32, tag="t4")
                    nc.vector.tensor_add(t4[:, :gn, :], B4[:, :gn, :],
                                         prior4_sb[:, :gn, :])
                    src4 = t4
                nc.vector.reduce_max(mg_all[:, g0:g0 + gn], src4[:, :gn, :],
                                     axis=AX, negate=True)
                e4 = work.tile([128, 8, K], mybir.dt.bfloat16, tag="e4")
                if F_BEXP:
                    # subtract the per-block max on DVE (stride-0 broadcast of
                    # -max along k), then ONE exp instruction per quad
                    t4 = work.tile([128, 8, K], mybir.dt.float32, tag="t4")
                    mgb = mg_all[:, g0:g0 + gn, None].broadcast_to(
                        [128, gn, K])
                    addeng = nc.gpsimd if ADDE == "g" else nc.vector
                    addeng.tensor_add(t4[:, :gn, :], src4[:, :gn, :], mgb)
                    nc.scalar.activation(e4[:, :gn, :], t4[:, :gn, :], AF.Exp)
                else:
                    for j in range(gn):
                        nc.scalar.activation(e4[:, j, :], src4[:, j, :], AF.Exp,
                                             bias=mg_all[:, g0 + j:g0 + j + 1],
                                             scale=1.0)
                with nc.allow_low_precision("bf16 sum-exp; ln() needs ~1e-2"):
                    nc.vector.reduce_sum(sg_all[:, g0:g0 + gn],
                                         e4[:, :gn, :], axis=AX)
                if g0 + gn in seg_after:
                    emit_seg(seg_after[g0 + gn])

            accp = const.tile([128, 1], f32)
            nc.vector.reduce_sum(accp[:], acc8[:], axis=AX)
            fin_ps = fps_pool.tile([1, 1], f32)
            nc.tensor.matmul(fin_ps[:], lhsT=accp[:], rhs=ones_sb[:],
                             start=True, stop=True)
            fin_sb = const.tile([1, 1], f32)
            nc.scalar.copy(fin_sb[:], fin_ps[:])
            nc.sync.dma_start(out=out, in_=fin_sb[:])

    nc.finalize()
    return nc


def _prep_inputs(counts, otu_dist, comm_dist, use_fp8):
    np_dt = ml_dtypes.float8_e4m3 if use_fp8 else ml_dtypes.bfloat16
    log_otu = np.log(otu_dist.astype(np.float32) + np.float32(EPS))
    hi = log_otu.astype(np_dt)
    # [p, c*64+k] = log_otu[k, c*128 + p]
    hi8 = np.ascontiguousarray(
        hi.reshape(K, 8, BLK).transpose(2, 1, 0)).reshape(128, 8 * K)

    prior_vec = np.log(comm_dist.astype(np.float32) + np.float32(EPS))
    # cancel the expected per-community weight-quantization bias
    # E_n[counts_n . delta_k] by folding it into the prior
    delta = hi.astype(np.float32) - log_otu            # (K, O)
    cbar = counts.astype(np.float32).mean(axis=0)      # (O,)
    prior_vec = prior_vec - delta @ cbar
    prior = np.ascontiguousarray(
        np.tile(prior_vec, 8)[None, :]).astype(np_dt)
    prior = prior.reshape(1, 8, K)

    counts_q = counts.astype(np_dt)
    shards = []
    for i in range(CORES):
        sh = counts_q[i * NSHARD:(i + 1) * NSHARD]
        shp = np.zeros((NPAD, O), np_dt)
        shp[:NSHARD] = sh
        # (s, b, j, c, p) -> (s, p, b, c, j)
        arr = shp.reshape(SBS, BPS, BLK, 8, BLK).transpose(0, 4, 1, 3, 2)
        shards.append(np.ascontiguousarray(arr).reshape(SBS, 128,
                                                        BPS * 8 * BLK))

    import os
    in_maps = [
        {"cnts": shards[i], "hi8": hi8, "prior": prior}
        for i in range(CORES)
    ]
    if os.environ.get("K_PRIORMM", "1") != "1":
        prior4 = np.ascontiguousarray(
            np.broadcast_to(np.tile(prior_vec.astype(np.float32), 8)[None, :],
                            (128, 8 * K))).reshape(128, 8, K)
        for m in in_maps:
            m["prior4"] = prior4
    # per-particle value contributed by each all-zero padding row; match
    # the device arithmetic (prior quantized to the counts dtype)
    pad_prior = prior_vec.astype(np_dt).astype(np.float64)
    pad_val = _np_logsumexp(pad_prior)
    return in_maps, pad_val


def _np_logsumexp(v):
    m = np.max(v)
    return m + np.log(np.sum(np.exp(v - m)))


def kernel(counts, otu_dist, comm_dist):
    from concourse.bass_utils import run_bass_kernel_spmd

    counts = np.asarray(counts)
    fp8 = ml_dtypes.float8_e4m3
    use_fp8 = bool(
        np.array_equal(counts.astype(fp8).astype(np.float32),
                       counts.astype(np.float32)))

    key = ("nc", use_fp8)
    if key not in _cache:
        _cache[key] = _build_module(use_fp8)
    nc = _cache[key]

    in_maps, pad_val = _prep_inputs(counts, np.asarray(otu_dist),
                                    np.asarray(comm_dist), use_fp8)
    res = run_bass_kernel_spmd(nc, in_maps, list(range(CORES)))
    total = sum(float(res.results[c]["out"][0, 0]) for c in range(CORES))
    total -= CORES * PAD_ROWS * pad_val
    return np.float32(total)

